# revision 15
# baseline (speedup 1.0000x reference)
"""Trainium2 kernel for Conv2d_cd (central-difference conv, 3x3, theta=0.7).

Reference math:
    s = sum of 9 shifted views of reflect-padded x  (= 3x3 box filter, reflect pad)
    out = conv3x3_zeropad(s, W) - theta * conv1x1(s, W.sum((2,3)))
        = conv3x3_zeropad(s, W')     with W'[:,:,1,1] -= theta * W.sum((2,3))

Strategy (per NeuronCore, 8 cores data-parallel over batch 16 -> 2 images/core):
  - x is loaded ONCE into a resident SBUF tile as bf16 via SWDGE cast-DMA
    (gpsimd queue), in row chunks; strips read arbitrary row windows from it.
    No fp32 x in SBUF, no cast ops, no halo re-reads.
  - images stacked on SBUF partition halves: partitions 0:64 = img0 ch, 64:128 = img1 ch
  - H strips of R output rows; per strip:
      * DVE separable box filter, all bf16 (2x mode)
      * F strips: H-pair + H-center + V-pair + V-center adds (9 conv taps)
      * T strips: V-pair + V-center only; H-box folded into 15-tap weights
      * conv taps as K=64/M=64 matmuls packed 4-concurrent in PE quadrants via
        tile_position, accumulating in PSUM
      * ScalarE evacuates PSUM -> SBUF; strided DMAs store to DRAM on the
        gpsimd + sync rings (Scalar keeps only evac, loads keep gpsimd early)
"""

import os

import numpy as np
import ml_dtypes

import concourse.bass as bass
import concourse.bacc as bacc
import concourse.mybir as mybir
from concourse.tile import TileContext
from concourse.bass_utils import run_bass_kernel_spmd

THETA = 0.7
N_CORES = 8
B, C, H, W = 16, 64, 128, 128
BPC = B // N_CORES          # images per core = 2
WP = W + 4                  # padded width of s buffers (132)
XROWS = H + 4               # resident x tile rows (2 pad top, 2 pad bottom)

# strip plan: comma list of F/T + height, heights multiples of 8 summing to 128.
# F = unfolded (9 taps + DVE H-box), T = folded (15 taps, no H-box).
def _parse_plan():
    txt = os.environ.get("K_PLAN", "FTTTTFFF@16")
    if "@" in txt:
        fold, r = txt.split("@")
        return [(int(r), c == "T") for c in fold]
    plan = []
    for item in txt.split(","):
        item = item.strip()
        plan.append((int(item[1:]), item[0] == "T"))
    assert sum(r for r, _ in plan) == H, plan
    return plan

PLAN = _parse_plan()
F32 = mybir.dt.float32
BF16 = mybir.dt.bfloat16

PSUM_BUFS = int(os.environ.get("K_PSUM_BUFS", "8"))
S0_SPLIT = os.environ.get("K_S0_SPLIT", "1") == "1"
EVAC_DVE_FROM = int(os.environ.get("K_EVAC_DVE_FROM", "99"))  # strips >= this: half evac on DVE
STORE_Q = os.environ.get("K_STORE_Q", "gsgs")  # per-store engine: a=scalar, s=sync, g=gpsimd
SBUFS = int(os.environ.get("K_SBUFS", "3"))
OBUFS = int(os.environ.get("K_OBUFS", "2"))
HBUFS = int(os.environ.get("K_HBUFS", "2"))
LD0 = int(os.environ.get("K_LD0", "10"))  # first chunk-load split row


def _host_weights(Wnp: np.ndarray):
    """Compute W' and the packed tap weight matrix on host (tiny).

    Layout: wt[p, t*C + co] with p in 0..127 (both halves identical copies),
    taps 0..8 = w9 (ky*3+kx), taps 9..23 = w15 (ky*5+tx).
    """
    Wp = Wnp.astype(np.float64).copy()
    Wp[:, :, 1, 1] -= THETA * Wnp.astype(np.float64).sum(axis=(2, 3))
    w9 = Wp.transpose(1, 2, 3, 0).reshape(C, 9, C)
    w15 = np.zeros((C, 3, 5, C), np.float64)
    for ky in range(3):
        for tx in range(5):
            for kx in range(max(0, tx - 2), min(2, tx) + 1):
                w15[:, ky, tx, :] += Wp[:, :, ky, kx].T  # [ci, co]
    wt = np.concatenate([w9.reshape(C, 9 * C), w15.reshape(C, 15 * C)], axis=1)
    wt = np.concatenate([wt, wt], axis=0)  # both partition halves
    return np.ascontiguousarray(wt.astype(ml_dtypes.bfloat16))


def build():
    nc = bacc.Bacc("TRN2", target_bir_lowering=False, debug=False,
                   num_devices=N_CORES)
    x_d = nc.declare_dram_parameter("x", [BPC, C, H, W], F32, isOutput=False)
    wt_d = nc.declare_dram_parameter("wt", [128, 24 * C], BF16, isOutput=False)
    out_d = nc.declare_dram_parameter("out", [BPC, C, H, W], F32, isOutput=True)

    # partition-major views: (img, ch) -> 128 partitions
    x_pc = x_d.rearrange("i c h w -> (i c) h w")
    # out view for strided stores: [img, ch, g2(16), two(2), (four*w)(512)]
    out_v = out_d.rearrange("i c (g2 two four) w -> i c g2 two (four w)",
                            two=2, four=4)

    pam = os.environ.get("K_POOL_MODE", "queue")
    with TileContext(nc, pool_alloc_mode=pam) as tc:
        with (
            tc.tile_pool(name="wpool", bufs=1) as wpool,
            tc.tile_pool(name="xpool", bufs=1) as xpool,
            tc.tile_pool(name="hpool", bufs=HBUFS) as hpool,
            tc.tile_pool(name="spool", bufs=SBUFS) as spool,
            tc.tile_pool(name="opool", bufs=OBUFS) as opool,
            tc.tile_pool(name="psum", bufs=PSUM_BUFS, space="PSUM") as ppool,
        ):
            # --- weights: single DMA, both partition halves pre-duplicated ---
            wt_sb = wpool.tile([128, 24 * C], BF16)
            nc.scalar.dma_start(out=wt_sb[:], in_=wt_d[:])

            # --- resident bf16 x: slot r = x row r-2 (2 pad rows each end) ---
            xb = xpool.tile([128, XROWS * W], BF16, tag="xb")
            xb3 = xb.rearrange("p (r w) -> p r w", w=W)
            # edge fixups first so strip-0 compute waits only on its loads
            nc.vector.memset(xb3[:, 0:1, :], 0.0)
            nc.gpsimd.dma_start(out=xb3[:, 1:2, :], in_=x_pc[:, 1:2, :])
            nc.vector.memset(xb3[:, XROWS - 1:XROWS, :], 0.0)
            # chunk loads (SWDGE cast fp32->bf16): [0,LD0), [LD0,18), then 16s
            bounds = [0, LD0, 18] + [18 + 16 * k for k in range(1, 7)] + [H]
            for lo, hi in zip(bounds[:-1], bounds[1:]):
                nc.gpsimd.dma_start(out=xb3[:, lo + 2:hi + 2, :],
                                    in_=x_pc[:, lo:hi, :])
            # bottom reflect row (x row 128 -> x row 126), only needed by the
            # last strip: issue after the chunk loads
            nc.gpsimd.dma_start(out=xb3[:, XROWS - 2:XROWS - 1, :],
                                in_=x_pc[:, 126:127, :])

            r0 = 0
            for si, (RS, folded) in enumerate(PLAN):
                XR, SR = RS + 4, RS + 2
                NP = RS * W // 1024
                last = si == len(PLAN) - 1
                sl = r0  # xb slot of x row r0-2

                if not folded:
                    # ---- horizontal pair sums: t = x(w-1) + x(w+1), bf16 2x
                    tt = hpool.tile([128, XR * W], BF16, tag="tt", bufs=1)
                    t3 = tt.rearrange("p (r w) -> p r w", w=W)
                    if si == 0:
                        hh = XR // 2
                        nc.vector.tensor_add(out=t3[:, 0:hh, 1:127],
                                             in0=xb3[:, sl:sl + hh, 0:126],
                                             in1=xb3[:, sl:sl + hh, 2:128])
                        nc.vector.tensor_add(out=t3[:, hh:XR, 1:127],
                                             in0=xb3[:, sl + hh:sl + XR, 0:126],
                                             in1=xb3[:, sl + hh:sl + XR, 2:128])
                    else:
                        nc.vector.tensor_add(out=t3[:, :, 1:127],
                                             in0=xb3[:, sl:sl + XR, 0:126],
                                             in1=xb3[:, sl:sl + XR, 2:128])
                    nc.vector.tensor_scalar_mul(out=t3[:, :, 0:1],
                                                in0=xb3[:, sl:sl + XR, 1:2],
                                                scalar1=2.0)
                    nc.vector.tensor_scalar_mul(out=t3[:, :, 127:128],
                                                in0=xb3[:, sl:sl + XR, 126:127],
                                                scalar1=2.0)
                    # ---- full h-box: v = t + x
                    vt = hpool.tile([128, XR * W], BF16, tag="vt")
                    nc.vector.tensor_add(out=vt[:], in0=tt[:],
                                         in1=xb[:, sl * W:(sl + XR) * W])
                    v3 = vt.rearrange("p (r w) -> p r w", w=W)
                    vsrc, vs0 = v3, 0
                else:
                    vsrc, vs0 = xb3, sl

                # ---- vertical box (bf16, 2x): s[j] = v[j] + v[j+1] + v[j+2]
                ut = hpool.tile([128, SR * W], BF16, tag="ut", bufs=1)
                st = spool.tile([128, SR * WP], BF16, tag="st")
                s3 = st.rearrange("p (r c) -> p r c", c=WP)
                u3 = ut.rearrange("p (r w) -> p r w", w=W)
                if si == 0 and S0_SPLIT and not folded:
                    sha = SR // 2 + 1  # rows 0..sha-1 cover pair-0 taps
                    sta = spool.tile([128, sha * WP], BF16, tag="sta", bufs=1)
                    s3a = sta.rearrange("p (r c) -> p r c", c=WP)
                    nc.vector.tensor_add(out=u3[:, 0:sha, :],
                                         in0=vsrc[:, vs0:vs0 + sha, :],
                                         in1=vsrc[:, vs0 + 2:vs0 + sha + 2, :])
                    nc.vector.tensor_add(out=s3a[:, :, 2:130],
                                         in0=u3[:, 0:sha, :],
                                         in1=vsrc[:, vs0 + 1:vs0 + sha + 1, :])
                    nc.any.memset(s3a[:, :, 0:2], 0.0)
                    nc.any.memset(s3a[:, :, 130:132], 0.0)
                    nc.any.memset(s3a[:, 0:1, :], 0.0)
                    nc.vector.tensor_add(out=u3[:, sha:SR, :],
                                         in0=vsrc[:, vs0 + sha:vs0 + SR, :],
                                         in1=vsrc[:, vs0 + sha + 2:vs0 + SR + 2, :])
                    nc.vector.tensor_add(out=s3[:, sha - 2:SR, 2:130],
                                         in0=u3[:, sha - 2:SR, :],
                                         in1=vsrc[:, vs0 + sha - 1:vs0 + SR + 1, :])
                else:
                    s3a = None
                    nc.vector.tensor_add(out=u3[:, :, :],
                                         in0=vsrc[:, vs0:vs0 + SR, :],
                                         in1=vsrc[:, vs0 + 2:vs0 + SR + 2, :])
                    nc.vector.tensor_add(out=s3[:, :, 2:130],
                                         in0=u3[:, :, :],
                                         in1=vsrc[:, vs0 + 1:vs0 + SR + 1, :])

                # ---- side columns
                if not folded:
                    # zero-pad columns for the 3-wide taps (cols 1..130 read)
                    nc.any.memset(s3[:, :, 0:2], 0.0)
                    nc.any.memset(s3[:, :, 130:132], 0.0)
                else:
                    # sv buffer: col c = sv[x-col c-2]; taps read cols 0..131.
                    # col1 := col3 (reflect -1 -> +1), col130 := col128
                    nc.vector.tensor_copy(out=s3[:, :, 1:2], in_=s3[:, :, 3:4])
                    nc.vector.tensor_copy(out=s3[:, :, 130:131], in_=s3[:, :, 128:129])
                    # col0 := -(col3 + col2)   [makes folded s(-1) == 0]
                    nc.vector.tensor_add(out=s3[:, :, 0:1],
                                         in0=s3[:, :, 3:4], in1=s3[:, :, 2:3])
                    nc.scalar.mul(s3[:, :, 0:1], s3[:, :, 0:1], -1.0)
                    # col131 := -(col129 + col128)  [makes folded s(128) == 0]
                    nc.vector.tensor_add(out=s3[:, :, 131:132],
                                         in0=s3[:, :, 129:130], in1=s3[:, :, 128:129])
                    nc.scalar.mul(s3[:, :, 131:132], s3[:, :, 131:132], -1.0)

                # ---- zero-pad rows (conv zero padding at image top/bottom)
                if si == 0:
                    nc.any.memset(s3[:, 0:1, :], 0.0)
                if last:
                    nc.any.memset(s3[:, SR - 1:SR, :], 0.0)

                # ---- conv taps: accumulate into 2*NP psum banks
                # chunk c = out local rows [4c, 4c+4); pair p = c//2
                # PA[p]: img0@0:64 (c even), img1@64:128 ; PB[p]: img0@64:128 (odd), img1@0:64
                pa = [ppool.tile([128, 512], F32, tag="ps", name=f"pa{si}_{j}")
                      for j in range(NP)]
                pb = [ppool.tile([128, 512], F32, tag="ps", name=f"pb{si}_{j}")
                      for j in range(NP)]
                ntap = 15 if folded else 9
                tap0 = 9 if folded else 0
                nkx = 5 if folded else 3
                cofs = 0 if folded else 1
                for p in range(NP):
                    for t in range(ntap):
                        ky, kx = t // nkx, t % nkx
                        tw = tap0 + t
                        for (i, c) in ((0, 2 * p), (1, 2 * p), (0, 2 * p + 1),
                                       (1, 2 * p + 1)):
                            ptile = pa[p] if c % 2 == 0 else pb[p]
                            pbase = 64 * i if c % 2 == 0 else 64 * (1 - i)
                            # s local row j = (out local row) + ky ; out local = 4c..4c+4
                            smat = s3a if (s3a is not None and p == 0) else s3
                            rhs = smat[64 * i:64 * i + 64,
                                       4 * c + ky:4 * c + ky + 4,
                                       kx + cofs:kx + cofs + 128]
                            nc.tensor.matmul(
                                ptile[pbase:pbase + 64, :],
                                wt_sb[64 * i:64 * i + 64, tw * C:(tw + 1) * C],
                                rhs,
                                start=(t == 0), stop=(t == ntap - 1),
                                skip_group_check=True,
                            )

                # ---- evacuate psum -> sbuf (ScalarE; DVE for late strips)
                ot = opool.tile([128, RS * W], F32, tag="ot")
                o3 = ot.rearrange("p (c n) -> p c n", n=512)
                for c in range(2 * NP):
                    ptile = pa[c // 2] if c % 2 == 0 else pb[c // 2]
                    if si >= EVAC_DVE_FROM and c % 2 == 1:
                        nc.vector.tensor_copy(out=o3[:, c:c + 1, :], in_=ptile[:])
                    else:
                        nc.scalar.copy(out=o3[:, c:c + 1, :], in_=ptile[:])

                # ---- store: 4 strided DMAs (even/odd chunks x partition halves)
                o4 = ot.rearrange("p (c2 two n) -> p c2 two n", two=2, n=512)
                g = r0 * W // 1024
                qmap = {"a": nc.scalar, "g": nc.gpsimd, "s": nc.sync}
                stores = [
                    (out_v[0, :, g:g + NP, 0, :], o4[0:64, :, 0, :]),
                    (out_v[0, :, g:g + NP, 1, :], o4[64:128, :, 1, :]),
                    (out_v[1, :, g:g + NP, 0, :], o4[64:128, :, 0, :]),
                    (out_v[1, :, g:g + NP, 1, :], o4[0:64, :, 1, :]),
                ]
                for (dst, srcp), qc in zip(stores, STORE_Q):
                    qmap[qc].dma_start(out=dst, in_=srcp)
                r0 += RS

    nc.compile()
    return nc


_CACHE = {}


def _get_nc():
    if "nc" not in _CACHE:
        _CACHE["nc"] = build()
    return _CACHE["nc"]


def kernel(x: np.ndarray, W: np.ndarray, trace: bool = False):
    x = np.asarray(x, dtype=np.float32)
    wt = _host_weights(np.asarray(W, dtype=np.float32))
    nc = _get_nc()
    core_ids = list(range(N_CORES))
    in_maps = [
        {"x": np.ascontiguousarray(x[BPC * i:BPC * (i + 1)]), "wt": wt}
        for i in core_ids
    ]
    res = run_bass_kernel_spmd(nc, in_maps, core_ids, trace=trace)
    out = np.concatenate([res.results[i]["out"] for i in core_ids], axis=0)
    if trace:
        kernel.last_exec_time_ns = res.exec_time_ns
        kernel.last_res = res
    return out


kernel.last_exec_time_ns = None


# revision 16
# speedup vs baseline: 1.1876x; 1.1876x over previous
"""Trainium2 kernel for Conv2d_cd (central-difference conv, 3x3, theta=0.7).

Reference math:
    s = sum of 9 shifted views of reflect-padded x  (= 3x3 box filter, reflect pad)
    out = conv3x3_zeropad(s, W) - theta * conv1x1(s, W.sum((2,3)))
        = conv3x3_zeropad(s, W')     with W'[:,:,1,1] -= theta * W.sum((2,3))

Strategy (per NeuronCore, 8 cores data-parallel over batch 16 -> 2 images/core):
  - images stacked on SBUF partition halves: partitions 0:64 = img0 ch, 64:128 = img1 ch
  - H strips of R output rows; per strip:
      * DVE separable box filter: horizontal pass fp32 (1x), vertical pass bf16 (2x)
        (folded strips skip the horizontal pass; it is folded into 15-tap weights)
      * conv taps as K=64/M=64 matmuls packed 4-concurrent in PE quadrants via
        tile_position (auto-derived from base partitions), accumulating in PSUM
      * ScalarE evacuates PSUM -> SBUF; strided DMAs store to DRAM
"""

import os

import numpy as np
import ml_dtypes

import concourse.bass as bass
import concourse.bacc as bacc
import concourse.mybir as mybir
from concourse.tile import TileContext
from concourse.bass_utils import run_bass_kernel_spmd

THETA = 0.7
N_CORES = 8
B, C, H, W = 16, 64, 128, 128
BPC = B // N_CORES          # images per core = 2
R = int(os.environ.get("K_R", "16"))        # strip height (output rows)
NSTRIP = H // R
WP = W + 4                  # padded width of s/sv buffers (132)

# strip plan: comma list of F/T + height, e.g. "F8,F8,T16,...". Heights are
# multiples of 8 summing to 128. F = unfolded (9 taps + DVE H-box), T = folded.
def _parse_plan():
    txt = os.environ.get("K_PLAN", "")
    if not txt:
        fold = os.environ.get("K_FOLDED", "")
        if len(fold) != NSTRIP:
            fold = "FTTTTFFF" if NSTRIP == 8 else "".join(
                "T" if si % 2 else "F" for si in range(NSTRIP))
        return [(R, c == "T") for c in fold]
    plan = []
    for item in txt.split(","):
        item = item.strip()
        plan.append((int(item[1:]), item[0] == "T"))
    assert sum(r for r, _ in plan) == H, plan
    return plan

PLAN = _parse_plan()
F32 = mybir.dt.float32
BF16 = mybir.dt.bfloat16

# which strips use the folded (15-tap, no horizontal DVE pass) path
_fold_env = os.environ.get("K_FOLDED", "")
if len(_fold_env) == NSTRIP:
    FOLDED = [c == "T" for c in _fold_env]
elif NSTRIP == 8:
    FOLDED = [c == "T" for c in "FTTTTFFF"]
else:
    FOLDED = [(si % 2 == 1) for si in range(NSTRIP)]

PSUM_BUFS = int(os.environ.get("K_PSUM_BUFS", "8"))
GP_H1 = os.environ.get("K_GP_H1", "0") == "1"
CAST_HOIST = int(os.environ.get("K_CAST_HOIST", "150"))
S0_SPLIT = os.environ.get("K_S0_SPLIT", "0") == "1"
EVAC_DVE_FROM = int(os.environ.get("K_EVAC_DVE_FROM", "6"))  # strips >= this: half evac on DVE
EVAC_ENG = os.environ.get("K_EVAC", "a")  # primary evac engine: a=scalar, g=gpsimd(broken: no psum read)
STORE_Q = os.environ.get("K_STORE_Q", "agsg")  # per-store engine: a=scalar, g=gpsimd, s=sync
XBUFS = int(os.environ.get("K_XBUFS", "4"))
SBUFS = int(os.environ.get("K_SBUFS", "3"))
OBUFS = int(os.environ.get("K_OBUFS", "2"))
HBUFS = int(os.environ.get("K_HBUFS", "2"))


def _host_weights(Wnp: np.ndarray):
    """Compute W' and the packed tap weight matrix on host (tiny).

    Layout: wt[p, t*C + co] with p in 0..127 (both halves identical copies),
    taps 0..8 = w9 (ky*3+kx), taps 9..23 = w15 (ky*5+tx).
    """
    Wp = Wnp.astype(np.float64).copy()
    Wp[:, :, 1, 1] -= THETA * Wnp.astype(np.float64).sum(axis=(2, 3))
    w9 = Wp.transpose(1, 2, 3, 0).reshape(C, 9, C)
    w15 = np.zeros((C, 3, 5, C), np.float64)
    for ky in range(3):
        for tx in range(5):
            for kx in range(max(0, tx - 2), min(2, tx) + 1):
                w15[:, ky, tx, :] += Wp[:, :, ky, kx].T  # [ci, co]
    wt = np.concatenate([w9.reshape(C, 9 * C), w15.reshape(C, 15 * C)], axis=1)
    wt = np.concatenate([wt, wt], axis=0)  # both partition halves
    return np.ascontiguousarray(wt.astype(ml_dtypes.bfloat16))


def build():
    nc = bacc.Bacc("TRN2", target_bir_lowering=False, debug=False,
                   num_devices=N_CORES)
    x_d = nc.declare_dram_parameter("x", [BPC, C, H, W], F32, isOutput=False)
    wt_d = nc.declare_dram_parameter("wt", [128, 24 * C], BF16, isOutput=False)
    out_d = nc.declare_dram_parameter("out", [BPC, C, H, W], F32, isOutput=True)

    # partition-major views: (img, ch) -> 128 partitions
    x_pc = x_d.rearrange("i c h w -> (i c) h w")
    # out view for strided stores: [img, ch, g2(16), two(2), (four*w)(512)]
    out_v = out_d.rearrange("i c (g2 two four) w -> i c g2 two (four w)",
                            two=2, four=4)

    pam = os.environ.get("K_POOL_MODE", "queue")
    with TileContext(nc, pool_alloc_mode=pam) as tc:
        with (
            tc.tile_pool(name="wpool", bufs=1) as wpool,
            tc.tile_pool(name="xpool", bufs=XBUFS) as xpool,
            tc.tile_pool(name="hpool", bufs=HBUFS) as hpool,
            tc.tile_pool(name="spool", bufs=SBUFS) as spool,
            tc.tile_pool(name="opool", bufs=OBUFS) as opool,
            tc.tile_pool(name="psum", bufs=PSUM_BUFS, space="PSUM") as ppool,
        ):
            # --- weights: single DMA, both partition halves pre-duplicated ---
            wt_sb = wpool.tile([128, 24 * C], BF16)
            nc.scalar.dma_start(out=wt_sb[:], in_=wt_d[:])

            r0 = 0
            for si, (RS, folded) in enumerate(PLAN):
                XR, SR = RS + 4, RS + 2
                NP = RS * W // 1024
                last = si == len(PLAN) - 1

                # ---- load x rows [r0-2, r0+R+2) into XR slots (slot m = abs row r0-2+m)
                xt = xpool.tile([128, XR * W], F32, tag="xt")
                x3 = xt.rearrange("p (r w) -> p r w", w=W)
                row_lo, row_hi = max(0, r0 - 2), min(H, r0 + RS + 2)
                slot_lo = row_lo - (r0 - 2)
                if si == 0:
                    # edge fixups FIRST so the first H-op only waits on the first load
                    nc.any.memset(x3[:, 0:1, :], 0.0)
                    nc.sync.dma_start(out=x3[:, 1:2, :], in_=x_pc[:, 1:2, :])
                    # split the first load so compute can start at half-load
                    mid = row_lo + XR // 2
                    nc.sync.dma_start(out=x3[:, slot_lo:slot_lo + (mid - row_lo), :],
                                      in_=x_pc[:, row_lo:mid, :])
                    nc.sync.dma_start(
                        out=x3[:, slot_lo + (mid - row_lo):slot_lo + (row_hi - row_lo), :],
                        in_=x_pc[:, mid:row_hi, :])
                else:
                    nc.sync.dma_start(out=x3[:, slot_lo:slot_lo + (row_hi - row_lo), :],
                                      in_=x_pc[:, row_lo:row_hi, :])
                if last:
                    # slot XR-2 = abs row 128 -> reflect = x row 126 ; slot XR-1 unused
                    nc.sync.dma_start(out=x3[:, XR - 2:XR - 1, :],
                                      in_=x_pc[:, 126:127, :])
                    nc.any.memset(x3[:, XR - 1:XR, :], 0.0)

                if not folded:
                    # ---- horizontal box (fp32): sh = x(w-1)+x(w)+x(w+1), reflect at edges
                    tt = hpool.tile([128, XR * W], F32, tag="tt", bufs=1)
                    t3 = tt.rearrange("p (r w) -> p r w", w=W)
                    heng = nc.gpsimd if GP_H1 else nc.vector
                    if si == 0:
                        hh = XR // 2
                        heng.tensor_add(out=t3[:, 0:hh, 1:127],
                                        in0=x3[:, 0:hh, 0:126],
                                        in1=x3[:, 0:hh, 2:128])
                        heng.tensor_add(out=t3[:, hh:XR, 1:127],
                                        in0=x3[:, hh:XR, 0:126],
                                        in1=x3[:, hh:XR, 2:128])
                    else:
                        heng.tensor_add(out=t3[:, :, 1:127],
                                        in0=x3[:, :, 0:126], in1=x3[:, :, 2:128])
                    nc.vector.tensor_scalar_mul(out=t3[:, :, 0:1],
                                                in0=x3[:, :, 1:2], scalar1=2.0)
                    nc.vector.tensor_scalar_mul(out=t3[:, :, 127:128],
                                                in0=x3[:, :, 126:127], scalar1=2.0)
                    vt = hpool.tile([128, XR * W], BF16, tag="vt")
                    nc.vector.tensor_add(out=vt[:], in0=tt[:], in1=xt[:])
                else:
                    # ---- bf16 cast of x on ScalarE (keeps both V-box ops in 2x mode)
                    # high_priority hoists it ahead of the previous strip's PSUM
                    # evacuation in the ScalarE stream (else DVE starves on it)
                    vt = hpool.tile([128, XR * W], BF16, tag="vt")
                    with tc.high_priority(offset=CAST_HOIST):
                        nc.scalar.copy(out=vt[:], in_=xt[:])
                v3 = vt.rearrange("p (r w) -> p r w", w=W)

                # ---- vertical box (bf16, 2x): s[j] = v[j] + v[j+1] + v[j+2]
                ut = hpool.tile([128, SR * W], BF16, tag="ut", bufs=1)
                st = spool.tile([128, SR * WP], BF16, tag="st")
                s3 = st.rearrange("p (r c) -> p r c", c=WP)
                u3 = ut.rearrange("p (r w) -> p r w", w=W)
                if si == 0 and S0_SPLIT:
                    # separate half-tile so pair-0 matmuls see their rows early
                    sha = SR // 2 + 1  # rows 0..sha-1 cover pair-0 taps
                    sta = spool.tile([128, sha * WP], BF16, tag="sta", bufs=1)
                    s3a = sta.rearrange("p (r c) -> p r c", c=WP)
                    nc.vector.tensor_add(out=ut[:, 0:sha * W],
                                         in0=vt[:, 0:sha * W],
                                         in1=vt[:, 2 * W:(sha + 2) * W])
                    nc.vector.tensor_add(out=s3a[:, :, 2:130],
                                         in0=u3[:, 0:sha, :], in1=v3[:, 1:sha + 1, :])
                    nc.any.memset(s3a[:, :, 0:2], 0.0)
                    nc.any.memset(s3a[:, :, 130:132], 0.0)
                    nc.any.memset(s3a[:, 0:1, :], 0.0)
                    nc.vector.tensor_add(out=ut[:, sha * W:SR * W],
                                         in0=vt[:, sha * W:SR * W],
                                         in1=vt[:, (sha + 2) * W:XR * W])
                    # overlap rows (sha-2..sha) also into s3: pair-1 taps read them
                    nc.vector.tensor_add(out=s3[:, sha - 2:SR, 2:130],
                                         in0=u3[:, sha - 2:SR, :],
                                         in1=v3[:, sha - 1:SR + 1, :])
                else:
                    s3a = None
                    nc.vector.tensor_add(out=ut[:], in0=vt[:, 0:SR * W],
                                         in1=vt[:, 2 * W:XR * W])
                    nc.vector.tensor_add(out=s3[:, :, 2:130],
                                         in0=u3[:, :, :], in1=v3[:, 1:SR + 1, :])

                # ---- side columns
                if not folded:
                    # zero-pad columns for the 3-wide taps (cols 1..130 read)
                    nc.any.memset(s3[:, :, 0:2], 0.0)
                    nc.any.memset(s3[:, :, 130:132], 0.0)
                else:
                    # sv buffer: col c = sv[x-col c-2]; taps read cols 0..131.
                    # col1 := col3 (reflect -1 -> +1), col130 := col128 (reflect 128 -> 126)
                    nc.vector.tensor_copy(out=s3[:, :, 1:2], in_=s3[:, :, 3:4])
                    nc.vector.tensor_copy(out=s3[:, :, 130:131], in_=s3[:, :, 128:129])
                    # col0 := -(col3 + col2)   [makes folded s(-1) == 0]
                    nc.vector.tensor_add(out=s3[:, :, 0:1],
                                         in0=s3[:, :, 3:4], in1=s3[:, :, 2:3])
                    nc.scalar.mul(s3[:, :, 0:1], s3[:, :, 0:1], -1.0)
                    # col131 := -(col129 + col128)  [makes folded s(128) == 0]
                    nc.vector.tensor_add(out=s3[:, :, 131:132],
                                         in0=s3[:, :, 129:130], in1=s3[:, :, 128:129])
                    nc.scalar.mul(s3[:, :, 131:132], s3[:, :, 131:132], -1.0)

                # ---- zero-pad rows (conv zero padding at image top/bottom)
                if si == 0:
                    nc.any.memset(s3[:, 0:1, :], 0.0)
                if last:
                    nc.any.memset(s3[:, SR - 1:SR, :], 0.0)

                # ---- conv taps: accumulate into 2*NP psum banks
                # chunk c = out local rows [4c, 4c+4); pair p = c//2
                # PA[p]: img0@0:64 (c even), img1@64:128 ; PB[p]: img0@64:128 (odd), img1@0:64
                pa = [ppool.tile([128, 512], F32, tag="ps", name=f"pa{si}_{j}")
                      for j in range(NP)]
                pb = [ppool.tile([128, 512], F32, tag="ps", name=f"pb{si}_{j}")
                      for j in range(NP)]
                ntap = 15 if folded else 9
                tap0 = 9 if folded else 0
                nkx = 5 if folded else 3
                cofs = 0 if folded else 1
                for p in range(NP):
                    for t in range(ntap):
                        ky, kx = t // nkx, t % nkx
                        tw = tap0 + t
                        for (i, c) in ((0, 2 * p), (1, 2 * p), (0, 2 * p + 1),
                                       (1, 2 * p + 1)):
                            ptile = pa[p] if c % 2 == 0 else pb[p]
                            pbase = 64 * i if c % 2 == 0 else 64 * (1 - i)
                            # s local row j = (out local row) + ky ; out local = 4c..4c+4
                            smat = s3a if (s3a is not None and p == 0) else s3
                            rhs = smat[64 * i:64 * i + 64,
                                       4 * c + ky:4 * c + ky + 4,
                                       kx + cofs:kx + cofs + 128]
                            nc.tensor.matmul(
                                ptile[pbase:pbase + 64, :],
                                wt_sb[64 * i:64 * i + 64, tw * C:(tw + 1) * C],
                                rhs,
                                start=(t == 0), stop=(t == ntap - 1),
                                skip_group_check=True,
                            )

                # ---- evacuate psum -> sbuf (ScalarE)
                ot = opool.tile([128, RS * W], F32, tag="ot")
                o3 = ot.rearrange("p (c n) -> p c n", n=512)
                for c in range(2 * NP):
                    ptile = pa[c // 2] if c % 2 == 0 else pb[c // 2]
                    if si >= EVAC_DVE_FROM and c % 2 == 1:
                        nc.vector.tensor_copy(out=o3[:, c:c + 1, :], in_=ptile[:])
                    elif EVAC_ENG == "g":
                        nc.gpsimd.tensor_copy(out=o3[:, c:c + 1, :], in_=ptile[:])
                    else:
                        nc.scalar.copy(out=o3[:, c:c + 1, :], in_=ptile[:])

                # ---- store: 4 strided DMAs (even/odd chunks x partition halves)
                # issue from two queues (ScalarE HWDGE + GpSimd SWDGE) to spread cost
                o4 = ot.rearrange("p (c2 two n) -> p c2 two n", two=2, n=512)
                g = r0 * W // 1024
                qmap = {"a": nc.scalar, "g": nc.gpsimd, "s": nc.sync}
                stores = [
                    (out_v[0, :, g:g + NP, 0, :], o4[0:64, :, 0, :]),
                    (out_v[0, :, g:g + NP, 1, :], o4[64:128, :, 1, :]),
                    (out_v[1, :, g:g + NP, 0, :], o4[64:128, :, 0, :]),
                    (out_v[1, :, g:g + NP, 1, :], o4[0:64, :, 1, :]),
                ]
                for (dst, srcp), qc in zip(stores, STORE_Q):
                    qmap[qc].dma_start(out=dst, in_=srcp)
                r0 += RS

    nc.compile()
    return nc


_CACHE = {}


def _get_nc():
    if "nc" not in _CACHE:
        _CACHE["nc"] = build()
    return _CACHE["nc"]


def kernel(x: np.ndarray, W: np.ndarray, trace: bool = False):
    x = np.asarray(x, dtype=np.float32)
    wt = _host_weights(np.asarray(W, dtype=np.float32))
    nc = _get_nc()
    core_ids = list(range(N_CORES))
    in_maps = [
        {"x": np.ascontiguousarray(x[BPC * i:BPC * (i + 1)]), "wt": wt}
        for i in core_ids
    ]
    res = run_bass_kernel_spmd(nc, in_maps, core_ids, trace=trace)
    out = np.concatenate([res.results[i]["out"] for i in core_ids], axis=0)
    if trace:
        kernel.last_exec_time_ns = res.exec_time_ns
        kernel.last_res = res
    return out


kernel.last_exec_time_ns = None

